# revision 13
# baseline (speedup 1.0000x reference)
"""Trainium2 Bass kernel for nn_DocumentHead (retrieval head MLP).

Math (per batch row):
    align = <v_claim, v_doc> / (max(||v_claim||,eps) * max(||v_doc||,eps))
    div   = 1 - align ; tens = div^2
    h      = relu([h_final | align | div | tens] @ W1 + b1)
    shared = relu(h @ W2 + b2)
    out    = sigmoid(shared @ Wr + br)

Strategy: data-parallel over batch on 8 cores (2048 rows/core). The whole
MLP chain runs in transposed space (features on partitions, batch on the
free dim) so W1/W2/Wr act as the stationary (lhsT) matmul operand.

v3 changes vs v2: ALL layout work moved to the host. make_in_maps()
pre-transposes h_final into the [p, kt, m] SBUF layout, pre-casts
x/v_claim/v_doc/W1/W2/Wr to bf16, pre-transposes the biases, and splits
off the 3 extra-feature rows of W1 — so the device sees DMA-ready
tensors. This cuts per-core HBM traffic from ~92 MB (f32 loads + a
DRAM bf16-staging round-trip for the x transpose) to ~28 MB, removes
every on-device cast/transpose (PE runs only the compute matmuls), and
frees the gpsimd/DVE/ACT engines for the stats chain. The extras matmul
contracts over K=4 partitions instead of a zero-padded K=128.

v4 changes vs v3 (both shrink PE busy-time, the bottleneck at ~89%):
- The 4 per-group extras matmuls run CONCURRENTLY in the PE array via
  tile_position row-strips (stationary rows at partitions 0/32/64/96,
  feats replicated to those partition offsets by widening the per-sc
  transpose matmul's lhsT) — span ~225 ns instead of 4 serial 213 ns
  matmuls. 16 -> 4 extras slots per super-chunk.
- Stage 3 (out = sigmoid(Wr.T @ sT + br)) moves off the PE: DVE does
  sT * broadcast(Wr) and a 3-step jt-fold, and a single ones-vector
  matmul does the cross-partition column sum (1 PE matmul per
  super-chunk instead of 8).
"""

import numpy as np

P = 128
D = 2048
NCORES = 8
FREE = 512          # moving free dim / batch-chunk width
KT = D // P         # 16 k-tiles for stage 1 contraction
NT = D // P         # 16 n-tiles  (stage-1 output features)
J = D // 2          # 1024
JT = J // P         # 8 j-tiles  (stage-2 output features)
EPS = 1e-12

_cache = {}


def _build(bc, reps=1):
    """Build the per-core Bass program for bc batch rows.

    reps > 1 repeats the whole pipeline over the same inputs inside one
    NEFF — used only for timing (amortizes host dispatch overhead).
    """
    import concourse.bass as bass
    import concourse.tile as tile
    from concourse import bacc, mybir
    from concourse.masks import make_identity

    f32 = mybir.dt.float32
    bf16 = mybir.dt.bfloat16
    AF = mybir.ActivationFunctionType
    OP = mybir.AluOpType

    nsc = bc // FREE            # super-chunks (= batch chunks) per core
    nmt = FREE // P             # m-tiles per super-chunk (4)

    nc = bacc.Bacc(trn_type="TRN2", target_bir_lowering=False, debug=False)

    # host-packed inputs (see make_in_maps / pack_core)
    xh = nc.dram_tensor("xh", [nsc, P, KT, FREE], bf16, kind="ExternalInput").ap()
    vch = nc.dram_tensor("vch", [bc, D], bf16, kind="ExternalInput").ap()
    vdh = nc.dram_tensor("vdh", [bc, D], bf16, kind="ExternalInput").ap()
    w1h = nc.dram_tensor("w1h", [P, KT, D], bf16, kind="ExternalInput").ap()
    w2h = nc.dram_tensor("w2h", [P, KT, J], bf16, kind="ExternalInput").ap()
    exh = nc.dram_tensor("exh", [P, D], bf16, kind="ExternalInput").ap()
    b1t = nc.dram_tensor("b1t", [P, NT], f32, kind="ExternalInput").ap()
    b2t = nc.dram_tensor("b2t", [P, JT], f32, kind="ExternalInput").ap()
    wrt = nc.dram_tensor("wrt", [P, JT], f32, kind="ExternalInput").ap()
    brt = nc.dram_tensor("brt", [1, 1], f32, kind="ExternalInput").ap()
    out = nc.dram_tensor("out", [bc, 1], f32, kind="ExternalOutput").ap()

    with tile.TileContext(nc) as tc:
        with (
            tc.tile_pool(name="singles", bufs=1) as singles,
            tc.tile_pool(name="xt", bufs=2) as xt_pool,
            tc.tile_pool(name="ht", bufs=1) as ht_pool,
            tc.tile_pool(name="st", bufs=1) as st_pool,
            tc.tile_pool(name="stage", bufs=2) as stage,
            tc.tile_pool(name="stats", bufs=2) as stats,
            tc.tile_pool(name="psA", bufs=1, space="PSUM") as psA,
            tc.tile_pool(name="psB", bufs=2, space="PSUM") as psB,
            tc.tile_pool(name="psT", bufs=2, space="PSUM") as psT,
        ):
            ident = singles.tile([P, P], bf16)
            make_identity(nc, ident)
            b1sb = singles.tile([P, NT], f32)
            b2sb = singles.tile([P, JT], f32)
            wrsb = singles.tile([P, JT], f32)
            brsb = singles.tile([1, 1], f32)
            exsb = singles.tile([P, D], bf16)
            # rotating per-sc halves: finish(nxt) writes half nxt%2 while
            # sc's stage-1 extras read half sc%2; feats live (replicated) at
            # partition offsets 0/32/64/96 for the row-strip extras matmuls,
            # other rows stay zero
            featsT = singles.tile([P, 2, FREE], bf16)
            nc.vector.memset(featsT, 0.0)
            # Wr broadcast along the free dim ([p, jt, m] = Wr[jt*128+p]) for
            # the DVE stage-3, plus a ones column for the PE column-sum
            wrb = singles.tile([P, JT, FREE], bf16)
            ones1 = singles.tile([P, 1], bf16)
            nc.vector.memset(ones1, 1.0)
            w1sb = singles.tile([P, KT, D], bf16)
            w2sb = singles.tile([P, KT, J], bf16)

            def load_small_consts():
                nc.sync.dma_start(b1sb, b1t)
                nc.sync.dma_start(b2sb, b2t)
                nc.sync.dma_start(wrsb, wrt)
                nc.sync.dma_start(brsb, brt)
                nc.sync.dma_start(exsb, exh)
                onesw = singles.tile([P, FREE], bf16)
                nc.vector.memset(onesw, 1.0)
                for jt in range(JT):
                    nc.vector.tensor_scalar_mul(wrb[:, jt, :], onesw,
                                                wrsb[:, jt:jt + 1])

            def load_w1_q(kt, qc):
                # column-quarter load order: stage-1 quarter qc only reads
                # w1sb[:, kt, qc*512:(qc+1)*512], so streaming W1 in
                # quarter-column order unblocks each stage-1 quarter after
                # ~2.1 MB instead of the full 8.4 MB
                cols = slice(qc * FREE, (qc + 1) * FREE)
                nc.sync.dma_start(w1sb[:, kt, cols], w1h[:, kt, cols])

            def load_w2_h(kt, ch):
                # column-half order: stage-2 jt-chains 0-3 only read
                # w2sb[:, :, 0:512]
                cols = slice(ch * FREE, (ch + 1) * FREE)
                nc.sync.dma_start(w2sb[:, kt, cols], w2h[:, kt, cols])

            sc_state = {}

            def rowbase(sc):
                return (sc % nsc) * nmt

            def phaseA_start(sc):
                s = dict(
                    ccs=stats.tile([P, nmt], f32, tag="ccs", name=f"ccs{sc}"),
                    dds=stats.tile([P, nmt], f32, tag="dds", name=f"dds{sc}"),
                    cds=stats.tile([P, nmt], f32, tag="cds", name=f"cds{sc}"),
                    xt=xt_pool.tile([P, KT, FREE], bf16, tag="xt", name=f"xt{sc}"),
                )
                sc_state[sc] = s
                # one fat contiguous 2 MB load for the whole super-chunk's
                # pre-transposed x — issued a full super-chunk ahead
                nc.sync.dma_start(s["xt"], xh[sc % nsc])

            def phaseA_v_dma(sc, mt):
                # full-width 512 KB v loads, issued one stage-1 quarter ahead
                # of their compute so the (strict-FIFO, depth-8) ACT queue
                # never head-blocks on DMA data
                s = sc_state[sc]
                row = (rowbase(sc) + mt) * P
                vcf = stage.tile([P, D], bf16, tag=f"vcf{mt % 2}",
                                 name=f"vc{sc}_{mt}", bufs=1)
                nc.gpsimd.dma_start(vcf, vch[row:row + P, :])
                vdf = stage.tile([P, D], bf16, tag=f"vdf{mt % 2}",
                                 name=f"vd{sc}_{mt}", bufs=1)
                nc.gpsimd.dma_start(vdf, vdh[row:row + P, :])
                s[f"vc{mt}"] = vcf
                s[f"vd{mt}"] = vdf

            def phaseA_v_compute(sc, mt):
                # cosine stats for one m-tile: one DVE mult + one DVE reduce,
                # and the two norms fall out of the ACT Squares' accum_out —
                # no partial-sum staging at all
                # NOTE: tensor_tensor_reduce crashes TRN2 here (device
                # unrecoverable) — use mult + reduce_sum instead
                s = sc_state[sc]
                vcf, vdf = s[f"vc{mt}"], s[f"vd{mt}"]
                trash = stage.tile([P, D], bf16, tag="trash",
                                   name=f"tr{sc}_{mt}", bufs=1)
                nc.vector.tensor_mul(trash, vcf, vdf)
                nc.vector.reduce_sum(s["cds"][:, mt:mt + 1], trash,
                                     axis=mybir.AxisListType.X)
                # in-place squares (after the DVE read above)
                nc.scalar.activation(vcf, vcf, AF.Square,
                                     accum_out=s["ccs"][:, mt:mt + 1])
                nc.scalar.activation(vdf, vdf, AF.Square,
                                     accum_out=s["dds"][:, mt:mt + 1])

            def phaseA_finish(sc):
                # stats -> [align, div, tens] rows of featsT; the per-m-tile
                # transpose is a REGULAR matmul against the identity (keeps
                # the PE in its warm clock state, unlike transpose-mode).
                # The lhsT is widened to 99 columns with the 3 feats rows
                # replicated at free-offsets 0/32/64/96, so ONE matmul lands
                # feats at partition strips 0/32/64/96 of featsT — the four
                # row-strip extras matmuls of stage 1 read one strip each.
                s = sc_state[sc]
                ccs, dds, cds = s["ccs"], s["dds"], s["cds"]
                feats = stats.tile([P, nmt, 3], f32, tag="feats", name=f"ft{sc}")
                featsw = stats.tile([P, nmt, 99], bf16, tag="featsw",
                                    name=f"fw{sc}")
                if sc < 2:
                    # first touch of each of the two rotating buffers: zero
                    # the never-written columns (3-31, 35-63, ...) so the
                    # transpose matmul can't propagate NaN garbage into the
                    # zero-weighted rows of featsT
                    nc.vector.memset(featsw, 0.0)
                nc.scalar.activation(ccs, ccs, AF.Sqrt)
                nc.scalar.activation(dds, dds, AF.Sqrt)
                nc.vector.tensor_scalar_max(ccs, ccs, EPS)
                nc.vector.tensor_scalar_max(dds, dds, EPS)
                nc.vector.tensor_mul(ccs, ccs, dds)
                nc.vector.reciprocal(ccs, ccs)
                nc.vector.tensor_mul(feats[:, :, 0], cds, ccs)      # align
                nc.vector.tensor_scalar(feats[:, :, 1], feats[:, :, 0],
                                        -1.0, 1.0, OP.mult, OP.add)  # div
                nc.vector.tensor_mul(feats[:, :, 2], feats[:, :, 1],
                                     feats[:, :, 1])                 # tens
                for i in range(4):
                    nc.vector.tensor_copy(featsw[:, :, 32 * i:32 * i + 3],
                                          feats)
                for mt in range(nmt):
                    psf = psT.tile([99, P], f32, tag="tp", name=f"psf{sc}_{mt}")
                    nc.tensor.matmul(psf, featsw[:, mt, :], ident,
                                     start=True, stop=True)
                    nc.vector.tensor_copy(
                        featsT[0:99, sc % 2, mt * P:(mt + 1) * P], psf)

            # prologue: sc0 x tile + small consts first, then W1 in
            # column-quarter order (all kt of quarter 0 first, so stage-1
            # quarter q unblocks after (q+1)*2.1 MB), with the v loads
            # (separate gpsimd queue) interleaved, then W2
            phaseA_start(0)
            load_small_consts()
            for g in range(nmt):
                phaseA_v_dma(0, g)
                for kt in range(KT):
                    load_w1_q(kt, g)
                phaseA_v_compute(0, g)
            for ch in range(2):
                for kt in range(KT):
                    load_w2_h(kt, ch)
            phaseA_finish(0)

            total_sc = nsc * reps
            for sc in range(total_sc):
                nxt = sc + 1 if sc + 1 < total_sc else None
                if nxt is not None:
                    phaseA_start(nxt)
                mcols = slice((sc % nsc) * FREE, (sc % nsc + 1) * FREE)

                # ---- stage 1: hT[n, m] = relu(W1.T @ xT + extras + b1) ----
                # groups of 4 nt-chains; the 4 extras matmuls of a group run
                # CONCURRENTLY in the PE array via tile_position row-strips
                # (K=4 each at partitions 0/32/64/96) — ~225 ns total instead
                # of 4 serial 213 ns matmuls
                ht = ht_pool.tile([P, NT, FREE], bf16)
                xt = sc_state[sc]["xt"]
                for g in range(NT // 4):
                    pss = []
                    for i in range(4):
                        nt = 4 * g + i
                        ps = psA.tile([P, FREE], mybir.dt.float32,
                                      tag=f"ps1_{i}",
                                      name=f"ps1_{sc}_{nt}")
                        for kt in range(KT):
                            nc.tensor.matmul(
                                ps, w1sb[:, kt, nt * P:(nt + 1) * P],
                                xt[:, kt, :], start=(kt == 0), stop=False)
                        pss.append((nt, ps))
                    for i, (nt, ps) in enumerate(pss):
                        nc.tensor.matmul(
                            ps, exsb[32 * i:32 * i + 4, nt * P:(nt + 1) * P],
                            featsT[32 * i:32 * i + 4, sc % 2, :],
                            start=False, stop=True, tile_position=(32 * i, 0))
                    for nt, ps in pss:
                        nc.scalar.activation(ht[:, nt, :], ps, AF.Relu,
                                             bias=b1sb[:, nt:nt + 1])
                    # interleave next-sc input prep between stage-1 groups:
                    # all non-PE work (DMA + DVE/ACT stats) so the PE stream
                    # stays dense. v loads run one slot ahead of their stats.
                    if nxt is not None:
                        phaseA_v_dma(nxt, g)
                        if g >= 1:
                            phaseA_v_compute(nxt, g - 1)

                if nxt is not None:
                    phaseA_v_compute(nxt, nmt - 1)

                # ---- stage 2: sT[j, m] = relu(W2.T @ hT + b2) ----
                st = st_pool.tile([P, JT, FREE], bf16)
                for jt in range(JT):
                    ps = psB.tile([P, FREE], mybir.dt.float32, tag="ps2")
                    for nt in range(NT):
                        nc.tensor.matmul(ps, w2sb[:, nt, jt * P:(jt + 1) * P],
                                         ht[:, nt, :], start=(nt == 0),
                                         stop=(nt == NT - 1))
                    nc.scalar.activation(st[:, jt, :], ps, AF.Relu,
                                         bias=b2sb[:, jt:jt + 1])

                # next-sc stats wrap-up AFTER stage 2: its 4 tiny PE matmuls
                # land behind the stage-2 stream, by which time the DVE stats
                # chain has long finished — no PE stall
                if nxt is not None:
                    phaseA_finish(nxt)

                # ---- stage 3: out[m] = sigmoid(Wr.T @ sT + br) ----
                # DVE: prod = sT * broadcast(Wr), fold 8 jt-slices -> 1;
                # PE: single ones-vector matmul does the cross-partition sum
                prod = stats.tile([P, JT, FREE], bf16, tag="prod",
                                  name=f"pr{sc}", bufs=1)
                nc.vector.tensor_mul(prod, st, wrb)
                nc.vector.tensor_add(prod[:, 0:4, :], prod[:, 0:4, :],
                                     prod[:, 4:8, :])
                nc.vector.tensor_add(prod[:, 0:2, :], prod[:, 0:2, :],
                                     prod[:, 2:4, :])
                prodb = stats.tile([P, FREE], bf16, tag="prodb",
                                   name=f"pb{sc}", bufs=1)
                nc.vector.tensor_add(prodb, prod[:, 0, :], prod[:, 1, :])
                psd = psB.tile([1, FREE], mybir.dt.float32, tag="ps2")
                nc.tensor.matmul(psd, ones1, prodb, start=True, stop=True)
                osb = stats.tile([1, FREE], f32, tag="osb", name=f"osb{sc}",
                                 bufs=1)
                nc.scalar.activation(osb, psd, AF.Sigmoid, bias=brsb[0:1, 0:1])
                nc.sync.dma_start(
                    out.rearrange("m one -> one m")[:, mcols], osb)

    nc.compile()
    return nc


def get_nc(bc, reps=1):
    if (bc, reps) not in _cache:
        _cache[(bc, reps)] = _build(bc, reps)
    return _cache[(bc, reps)]


def _shim_axon_hooks():
    """antenv.axon_hooks is absent in this container; shim it so a
    BASS_TRACE=1 environment can't crash run_bass_kernel_spmd."""
    import sys
    import types
    try:
        import antenv
    except ImportError:
        return
    if "antenv.axon_hooks" not in sys.modules:
        try:
            import antenv.axon_hooks  # noqa: F401
        except ImportError:
            m = types.ModuleType("antenv.axon_hooks")
            m.get_axon_ntff_profile_hook = lambda: None
            sys.modules["antenv.axon_hooks"] = m
            antenv.axon_hooks = m


def batch_per_core(inputs):
    return np.asarray(inputs["h_final"]).shape[0] // NCORES


def pack_weights(inputs):
    """Host-side pack of the replicated (per-core-identical) tensors."""
    import ml_dtypes
    bf16 = ml_dtypes.bfloat16
    W1 = np.asarray(inputs["W1"], dtype=np.float32)
    W2 = np.asarray(inputs["W2"], dtype=np.float32)
    Wr = np.asarray(inputs["Wr"], dtype=np.float32)
    b1 = np.asarray(inputs["b1"], dtype=np.float32)
    b2 = np.asarray(inputs["b2"], dtype=np.float32)
    br = np.asarray(inputs["br"], dtype=np.float32)
    # extras weights replicated at partition strips 0/32/64/96 for the
    # tile_position row-strip matmuls; all other rows zero
    exh = np.zeros((P, D), dtype=bf16)
    for i in range(4):
        exh[32 * i:32 * i + 3] = W1[D:D + 3].astype(bf16)
    return {
        # [p, kt, n] = W1[kt*128+p, n]
        "w1h": np.ascontiguousarray(
            W1[:D].reshape(KT, P, D).transpose(1, 0, 2).astype(bf16)),
        "w2h": np.ascontiguousarray(
            W2.reshape(KT, P, J).transpose(1, 0, 2).astype(bf16)),
        "exh": exh,
        "b1t": np.ascontiguousarray(b1.reshape(NT, P).T),
        "b2t": np.ascontiguousarray(b2.reshape(JT, P).T),
        "wrt": np.ascontiguousarray(Wr[:, 0].reshape(JT, P).T),
        "brt": br.reshape(1, 1),
    }


def pack_core(hf, vc, vd):
    """Host-side pack of one core's batch slice.

    xh[sc, p, kt, f] = h_final[sc*FREE + f, kt*P + p]  (bf16) — the exact
    SBUF layout stage 1 consumes, so the device does a single contiguous
    2 MB DMA per super-chunk and no transposes at all.
    """
    import ml_dtypes
    bf16 = ml_dtypes.bfloat16
    bc = hf.shape[0]
    nsc = bc // FREE
    xh = np.ascontiguousarray(
        hf.reshape(nsc, FREE, KT, P).transpose(0, 3, 2, 1).astype(bf16))
    return {
        "xh": xh,
        "vch": np.ascontiguousarray(vc.astype(bf16)),
        "vdh": np.ascontiguousarray(vd.astype(bf16)),
    }


def make_in_maps(inputs):
    B = np.asarray(inputs["h_final"]).shape[0]
    bc = B // NCORES
    shared = pack_weights(inputs)
    hf = np.asarray(inputs["h_final"], dtype=np.float32)
    vc = np.asarray(inputs["v_claim"], dtype=np.float32)
    vd = np.asarray(inputs["v_doc"], dtype=np.float32)
    in_maps = []
    for c in range(NCORES):
        sl = slice(c * bc, (c + 1) * bc)
        m = dict(shared)
        m.update(pack_core(hf[sl], vc[sl], vd[sl]))
        in_maps.append(m)
    return in_maps


def kernel(**inputs):
    _shim_axon_hooks()
    from concourse.bass_utils import run_bass_kernel_spmd

    bc = batch_per_core(inputs)
    nc = get_nc(bc)
    in_maps = make_in_maps(inputs)
    res = run_bass_kernel_spmd(nc, in_maps, core_ids=list(range(NCORES)))
    return np.concatenate([r["out"] for r in res.results], axis=0)


# revision 18
# speedup vs baseline: 1.1085x; 1.1085x over previous
"""Trainium2 Bass kernel for nn_DocumentHead (retrieval head MLP).

Math (per batch row):
    align = <v_claim, v_doc> / (max(||v_claim||,eps) * max(||v_doc||,eps))
    div   = 1 - align ; tens = div^2
    h      = relu([h_final | align | div | tens] @ W1 + b1)
    shared = relu(h @ W2 + b2)
    out    = sigmoid(shared @ Wr + br)

Strategy: data-parallel over batch on 8 cores (2048 rows/core). The whole
MLP chain runs in transposed space (features on partitions, batch on the
free dim) so W1/W2/Wr act as the stationary (lhsT) matmul operand.

v3 changes vs v2: ALL layout work moved to the host. make_in_maps()
pre-transposes h_final into the [p, kt, m] SBUF layout, pre-casts
x/v_claim/v_doc/W1/W2/Wr to bf16, pre-transposes the biases, and splits
off the 3 extra-feature rows of W1 — so the device sees DMA-ready
tensors. This cuts per-core HBM traffic from ~92 MB (f32 loads + a
DRAM bf16-staging round-trip for the x transpose) to ~28 MB, removes
every on-device cast/transpose (PE runs only the compute matmuls), and
frees the gpsimd/DVE/ACT engines for the stats chain. The extras matmul
contracts over K=4 partitions instead of a zero-padded K=128.

v4 changes vs v3 (both shrink PE busy-time, the bottleneck at ~89%):
- The 4 per-group extras matmuls run CONCURRENTLY in the PE array via
  tile_position row-strips (stationary rows at partitions 0/32/64/96,
  feats replicated to those partition offsets by widening the per-sc
  transpose matmul's lhsT) — span ~225 ns instead of 4 serial 213 ns
  matmuls. 16 -> 4 extras slots per super-chunk.
- Stage 3 (out = sigmoid(Wr.T @ sT + br)) moves off the PE: DVE does
  sT * broadcast(Wr) and a 3-step jt-fold, and a single ones-vector
  matmul does the cross-partition column sum (1 PE matmul per
  super-chunk instead of 8).
"""

import numpy as np

P = 128
D = 2048
NCORES = 8
FREE = 512          # moving free dim / batch-chunk width
KT = D // P         # 16 k-tiles for stage 1 contraction
NT = D // P         # 16 n-tiles  (stage-1 output features)
J = D // 2          # 1024
JT = J // P         # 8 j-tiles  (stage-2 output features)
EPS = 1e-12

_cache = {}


def _build(bc, reps=1):
    """Build the per-core Bass program for bc batch rows.

    reps > 1 repeats the whole pipeline over the same inputs inside one
    NEFF — used only for timing (amortizes host dispatch overhead).
    """
    import concourse.bass as bass
    import concourse.tile as tile
    from concourse import bacc, mybir
    from concourse.masks import make_identity

    f32 = mybir.dt.float32
    bf16 = mybir.dt.bfloat16
    AF = mybir.ActivationFunctionType
    OP = mybir.AluOpType

    nsc = bc // FREE            # super-chunks (= batch chunks) per core
    nmt = FREE // P             # m-tiles per super-chunk (4)

    nc = bacc.Bacc(trn_type="TRN2", target_bir_lowering=False, debug=False)

    # host-packed inputs (see make_in_maps / pack_core)
    xh = nc.dram_tensor("xh", [nsc, P, KT, FREE], bf16, kind="ExternalInput").ap()
    vch = nc.dram_tensor("vch", [bc, D], bf16, kind="ExternalInput").ap()
    vdh = nc.dram_tensor("vdh", [bc, D], bf16, kind="ExternalInput").ap()
    w1h = nc.dram_tensor("w1h", [P, KT, D], bf16, kind="ExternalInput").ap()
    w2h = nc.dram_tensor("w2h", [P, KT, J], bf16, kind="ExternalInput").ap()
    exh = nc.dram_tensor("exh", [P, D], bf16, kind="ExternalInput").ap()
    b1t = nc.dram_tensor("b1t", [P, NT], f32, kind="ExternalInput").ap()
    b2t = nc.dram_tensor("b2t", [P, JT], f32, kind="ExternalInput").ap()
    wrt = nc.dram_tensor("wrt", [P, JT], f32, kind="ExternalInput").ap()
    brt = nc.dram_tensor("brt", [1, 1], f32, kind="ExternalInput").ap()
    out = nc.dram_tensor("out", [bc, 1], f32, kind="ExternalOutput").ap()

    with tile.TileContext(nc) as tc:
        with (
            tc.tile_pool(name="singles", bufs=1) as singles,
            tc.tile_pool(name="xt", bufs=2) as xt_pool,
            tc.tile_pool(name="ht", bufs=1) as ht_pool,
            tc.tile_pool(name="st", bufs=1) as st_pool,
            tc.tile_pool(name="stage", bufs=2) as stage,
            tc.tile_pool(name="stats", bufs=2) as stats,
            tc.tile_pool(name="psA", bufs=1, space="PSUM") as psA,
            tc.tile_pool(name="psB", bufs=2, space="PSUM") as psB,
            tc.tile_pool(name="psT", bufs=2, space="PSUM") as psT,
        ):
            ident = singles.tile([P, P], bf16)
            make_identity(nc, ident)
            b1sb = singles.tile([P, NT], f32)
            b2sb = singles.tile([P, JT], f32)
            wrsb = singles.tile([P, JT], f32)
            brsb = singles.tile([1, 1], f32)
            exsb = singles.tile([P, D], bf16)
            # rotating per-sc halves: finish(nxt) writes half nxt%2 while
            # sc's stage-1 extras read half sc%2; feats live (replicated) at
            # partition offsets 0/32/64/96 for the row-strip extras matmuls,
            # other rows stay zero
            featsT = singles.tile([P, 2, FREE], bf16)
            nc.vector.memset(featsT, 0.0)
            # Wr broadcast along the free dim ([p, jt, m] = Wr[jt*128+p]) for
            # the DVE stage-3, plus a ones column for the PE column-sum
            wrb = singles.tile([P, JT, FREE], bf16)
            ones1 = singles.tile([P, 1], bf16)
            nc.vector.memset(ones1, 1.0)
            w1sb = singles.tile([P, KT, D], bf16)
            w2sb = singles.tile([P, KT, J], bf16)

            def load_small_consts():
                nc.sync.dma_start(b1sb, b1t)
                nc.sync.dma_start(b2sb, b2t)
                nc.sync.dma_start(wrsb, wrt)
                nc.sync.dma_start(brsb, brt)
                nc.sync.dma_start(exsb, exh)
                onesw = singles.tile([P, FREE], bf16)
                nc.vector.memset(onesw, 1.0)
                for jt in range(JT):
                    nc.vector.tensor_scalar_mul(wrb[:, jt, :], onesw,
                                                wrsb[:, jt:jt + 1])

            def load_w1_q(kt, qc):
                # column-quarter load order: stage-1 quarter qc only reads
                # w1sb[:, kt, qc*512:(qc+1)*512], so streaming W1 in
                # quarter-column order unblocks each stage-1 quarter after
                # ~2.1 MB instead of the full 8.4 MB
                cols = slice(qc * FREE, (qc + 1) * FREE)
                nc.sync.dma_start(w1sb[:, kt, cols], w1h[:, kt, cols])

            def load_w2_h(kt, ch):
                # column-half order: stage-2 jt-chains 0-3 only read
                # w2sb[:, :, 0:512]
                cols = slice(ch * FREE, (ch + 1) * FREE)
                nc.sync.dma_start(w2sb[:, kt, cols], w2h[:, kt, cols])

            sc_state = {}

            def rowbase(sc):
                return (sc % nsc) * nmt

            def phaseA_start(sc):
                s = dict(
                    ccs=stats.tile([P, nmt], f32, tag="ccs", name=f"ccs{sc}"),
                    dds=stats.tile([P, nmt], f32, tag="dds", name=f"dds{sc}"),
                    cds=stats.tile([P, nmt], f32, tag="cds", name=f"cds{sc}"),
                    xt=xt_pool.tile([P, KT, FREE], bf16, tag="xt", name=f"xt{sc}"),
                )
                sc_state[sc] = s
                # one fat contiguous 2 MB load for the whole super-chunk's
                # pre-transposed x — issued a full super-chunk ahead
                nc.sync.dma_start(s["xt"], xh[sc % nsc])

            def phaseA_v_dma(sc, mt):
                # full-width 512 KB v loads, issued one stage-1 quarter ahead
                # of their compute so the (strict-FIFO, depth-8) ACT queue
                # never head-blocks on DMA data
                s = sc_state[sc]
                row = (rowbase(sc) + mt) * P
                vcf = stage.tile([P, D], bf16, tag=f"vcf{mt % 2}",
                                 name=f"vc{sc}_{mt}", bufs=1)
                nc.gpsimd.dma_start(vcf, vch[row:row + P, :])
                vdf = stage.tile([P, D], bf16, tag=f"vdf{mt % 2}",
                                 name=f"vd{sc}_{mt}", bufs=1)
                nc.gpsimd.dma_start(vdf, vdh[row:row + P, :])
                s[f"vc{mt}"] = vcf
                s[f"vd{mt}"] = vdf

            def phaseA_v_compute(sc, mt):
                # cosine stats for one m-tile: one DVE mult + one DVE reduce,
                # and the two norms fall out of the ACT Squares' accum_out —
                # no partial-sum staging at all
                # NOTE: tensor_tensor_reduce crashes TRN2 here (device
                # unrecoverable) — use mult + reduce_sum instead
                s = sc_state[sc]
                vcf, vdf = s[f"vc{mt}"], s[f"vd{mt}"]
                trash = stage.tile([P, D], bf16, tag="trash",
                                   name=f"tr{sc}_{mt}", bufs=1)
                nc.vector.tensor_mul(trash, vcf, vdf)
                nc.vector.reduce_sum(s["cds"][:, mt:mt + 1], trash,
                                     axis=mybir.AxisListType.X)
                # in-place squares (after the DVE read above)
                nc.scalar.activation(vcf, vcf, AF.Square,
                                     accum_out=s["ccs"][:, mt:mt + 1])
                nc.scalar.activation(vdf, vdf, AF.Square,
                                     accum_out=s["dds"][:, mt:mt + 1])

            def phaseA_finish(sc):
                # stats -> [align, div, tens] rows of featsT; the per-m-tile
                # transpose is a REGULAR matmul against the identity (keeps
                # the PE in its warm clock state, unlike transpose-mode).
                # The lhsT is widened to 99 columns with the 3 feats rows
                # replicated at free-offsets 0/32/64/96, so ONE matmul lands
                # feats at partition strips 0/32/64/96 of featsT — the four
                # row-strip extras matmuls of stage 1 read one strip each.
                s = sc_state[sc]
                ccs, dds, cds = s["ccs"], s["dds"], s["cds"]
                feats = stats.tile([P, nmt, 3], f32, tag="feats", name=f"ft{sc}")
                featsw = stats.tile([P, nmt, 99], bf16, tag="featsw",
                                    name=f"fw{sc}")
                if sc < 2:
                    # first touch of each of the two rotating buffers: zero
                    # the never-written columns (3-31, 35-63, ...) so the
                    # transpose matmul can't propagate NaN garbage into the
                    # zero-weighted rows of featsT
                    nc.vector.memset(featsw, 0.0)
                nc.scalar.activation(ccs, ccs, AF.Sqrt)
                nc.scalar.activation(dds, dds, AF.Sqrt)
                nc.vector.tensor_scalar_max(ccs, ccs, EPS)
                nc.vector.tensor_scalar_max(dds, dds, EPS)
                nc.vector.tensor_mul(ccs, ccs, dds)
                nc.vector.reciprocal(ccs, ccs)
                nc.vector.tensor_mul(feats[:, :, 0], cds, ccs)      # align
                nc.vector.tensor_scalar(feats[:, :, 1], feats[:, :, 0],
                                        -1.0, 1.0, OP.mult, OP.add)  # div
                nc.vector.tensor_mul(feats[:, :, 2], feats[:, :, 1],
                                     feats[:, :, 1])                 # tens
                for i in range(4):
                    nc.vector.tensor_copy(featsw[:, :, 32 * i:32 * i + 3],
                                          feats)
                for mt in range(nmt):
                    psf = psT.tile([99, P], f32, tag="tp", name=f"psf{sc}_{mt}")
                    nc.tensor.matmul(psf, featsw[:, mt, :], ident,
                                     start=True, stop=True)
                    nc.vector.tensor_copy(
                        featsT[0:99, sc % 2, mt * P:(mt + 1) * P], psf)

            # prologue: sc0 x tile + small consts first, then W1 in
            # column-quarter order (all kt of quarter 0 first, so stage-1
            # quarter q unblocks after (q+1)*2.1 MB), with the v loads
            # (separate gpsimd queue) interleaved, then W2
            phaseA_start(0)
            load_small_consts()
            for g in range(nmt):
                phaseA_v_dma(0, g)
                for kt in range(KT):
                    load_w1_q(kt, g)
                phaseA_v_compute(0, g)
            for ch in range(2):
                for kt in range(KT):
                    load_w2_h(kt, ch)
            phaseA_finish(0)

            def emit_stage3_tail(sc):
                # cross-partition column sum of the folded Wr products via a
                # single ones-vector matmul, then sigmoid + store
                s = sc_state[sc]
                psd = psB.tile([1, FREE], mybir.dt.float32, tag="ps2",
                               name=f"psd{sc}")
                nc.tensor.matmul(psd, ones1, s["prodb"], start=True, stop=True)
                osb = stats.tile([1, FREE], f32, tag="osb", name=f"osb{sc}",
                                 bufs=1)
                nc.scalar.activation(osb, psd, AF.Sigmoid, bias=brsb[0:1, 0:1])
                nc.sync.dma_start(
                    out.rearrange("m one -> one m")[:, s["mcols"]], osb)

            total_sc = nsc * reps
            for sc in range(total_sc):
                nxt = sc + 1 if sc + 1 < total_sc else None
                if nxt is not None:
                    phaseA_start(nxt)
                mcols = slice((sc % nsc) * FREE, (sc % nsc + 1) * FREE)

                # ---- stage 1: hT[n, m] = relu(W1.T @ xT + extras + b1) ----
                # groups of 4 nt-chains; the 4 extras matmuls of a group run
                # CONCURRENTLY in the PE array via tile_position row-strips
                # (K=4 each at partitions 0/32/64/96) — ~225 ns total instead
                # of 4 serial 213 ns matmuls
                ht = ht_pool.tile([P, NT, FREE], bf16)
                xt = sc_state[sc]["xt"]
                for g in range(NT // 4):
                    pss = []
                    for i in range(4):
                        nt = 4 * g + i
                        ps = psA.tile([P, FREE], mybir.dt.float32,
                                      tag=f"ps1_{i}",
                                      name=f"ps1_{sc}_{nt}")
                        for kt in range(KT):
                            nc.tensor.matmul(
                                ps, w1sb[:, kt, nt * P:(nt + 1) * P],
                                xt[:, kt, :], start=(kt == 0), stop=False)
                        pss.append((nt, ps))
                    for i, (nt, ps) in enumerate(pss):
                        nc.tensor.matmul(
                            ps, exsb[32 * i:32 * i + 4, nt * P:(nt + 1) * P],
                            featsT[32 * i:32 * i + 4, sc % 2, :],
                            start=False, stop=True, tile_position=(32 * i, 0))
                    for nt, ps in pss:
                        nc.scalar.activation(ht[:, nt, :], ps, AF.Relu,
                                             bias=b1sb[:, nt:nt + 1])
                    # interleave next-sc input prep between stage-1 groups:
                    # all non-PE work (DMA + DVE/ACT stats) so the PE stream
                    # stays dense. v loads run one slot ahead of their stats.
                    if nxt is not None:
                        phaseA_v_dma(nxt, g)
                        if g >= 1:
                            phaseA_v_compute(nxt, g - 1)
                    # previous super-chunk's deferred stage-3 tail: by now its
                    # DVE fold chain has long drained, so the ones-matmul
                    # slots into the PE stream with zero wait
                    if g == 0 and sc >= 1:
                        emit_stage3_tail(sc - 1)

                if nxt is not None:
                    phaseA_v_compute(nxt, nmt - 1)

                # ---- stage 2: sT[j, m] = relu(W2.T @ hT + b2) ----
                st = st_pool.tile([P, JT, FREE], bf16)
                for jt in range(JT):
                    ps = psB.tile([P, FREE], mybir.dt.float32, tag="ps2")
                    for nt in range(NT):
                        nc.tensor.matmul(ps, w2sb[:, nt, jt * P:(jt + 1) * P],
                                         ht[:, nt, :], start=(nt == 0),
                                         stop=(nt == NT - 1))
                    nc.scalar.activation(st[:, jt, :], ps, AF.Relu,
                                         bias=b2sb[:, jt:jt + 1])

                # next-sc stats wrap-up AFTER stage 2: its 4 tiny PE matmuls
                # land behind the stage-2 stream, by which time the DVE stats
                # chain has long finished — no PE stall
                if nxt is not None:
                    phaseA_finish(nxt)

                # ---- stage 3 (DVE part): prod = sT * broadcast(Wr), then
                # fold the 8 jt-slices down to 1. The PE column-sum + sigmoid
                # + store are DEFERRED into the next super-chunk's stage-1
                # (emit_stage3_tail) — issuing the ones-matmul here would
                # head-block the in-order PE queue on this ~2.5 us DVE chain.
                prod = stats.tile([P, JT, FREE], bf16, tag="prod",
                                  name=f"pr{sc}", bufs=1)
                nc.vector.tensor_mul(prod, st, wrb)
                nc.vector.tensor_add(prod[:, 0:4, :], prod[:, 0:4, :],
                                     prod[:, 4:8, :])
                nc.vector.tensor_add(prod[:, 0:2, :], prod[:, 0:2, :],
                                     prod[:, 2:4, :])
                prodb = stats.tile([P, FREE], bf16, tag="prodb",
                                   name=f"pb{sc}", bufs=1)
                nc.vector.tensor_add(prodb, prod[:, 0, :], prod[:, 1, :])
                sc_state[sc]["prodb"] = prodb
                sc_state[sc]["mcols"] = mcols
                if nxt is None:
                    emit_stage3_tail(sc)

    nc.compile()
    return nc


def get_nc(bc, reps=1):
    if (bc, reps) not in _cache:
        _cache[(bc, reps)] = _build(bc, reps)
    return _cache[(bc, reps)]


def _shim_axon_hooks():
    """antenv.axon_hooks is absent in this container; shim it so a
    BASS_TRACE=1 environment can't crash run_bass_kernel_spmd."""
    import sys
    import types
    try:
        import antenv
    except ImportError:
        return
    if "antenv.axon_hooks" not in sys.modules:
        try:
            import antenv.axon_hooks  # noqa: F401
        except ImportError:
            m = types.ModuleType("antenv.axon_hooks")
            m.get_axon_ntff_profile_hook = lambda: None
            sys.modules["antenv.axon_hooks"] = m
            antenv.axon_hooks = m


def batch_per_core(inputs):
    return np.asarray(inputs["h_final"]).shape[0] // NCORES


def pack_weights(inputs):
    """Host-side pack of the replicated (per-core-identical) tensors."""
    import ml_dtypes
    bf16 = ml_dtypes.bfloat16
    W1 = np.asarray(inputs["W1"], dtype=np.float32)
    W2 = np.asarray(inputs["W2"], dtype=np.float32)
    Wr = np.asarray(inputs["Wr"], dtype=np.float32)
    b1 = np.asarray(inputs["b1"], dtype=np.float32)
    b2 = np.asarray(inputs["b2"], dtype=np.float32)
    br = np.asarray(inputs["br"], dtype=np.float32)
    # extras weights replicated at partition strips 0/32/64/96 for the
    # tile_position row-strip matmuls; all other rows zero
    exh = np.zeros((P, D), dtype=bf16)
    for i in range(4):
        exh[32 * i:32 * i + 3] = W1[D:D + 3].astype(bf16)
    return {
        # [p, kt, n] = W1[kt*128+p, n]
        "w1h": np.ascontiguousarray(
            W1[:D].reshape(KT, P, D).transpose(1, 0, 2).astype(bf16)),
        "w2h": np.ascontiguousarray(
            W2.reshape(KT, P, J).transpose(1, 0, 2).astype(bf16)),
        "exh": exh,
        "b1t": np.ascontiguousarray(b1.reshape(NT, P).T),
        "b2t": np.ascontiguousarray(b2.reshape(JT, P).T),
        "wrt": np.ascontiguousarray(Wr[:, 0].reshape(JT, P).T),
        "brt": br.reshape(1, 1),
    }


def pack_core(hf, vc, vd):
    """Host-side pack of one core's batch slice.

    xh[sc, p, kt, f] = h_final[sc*FREE + f, kt*P + p]  (bf16) — the exact
    SBUF layout stage 1 consumes, so the device does a single contiguous
    2 MB DMA per super-chunk and no transposes at all.
    """
    import ml_dtypes
    bf16 = ml_dtypes.bfloat16
    bc = hf.shape[0]
    nsc = bc // FREE
    xh = np.ascontiguousarray(
        hf.reshape(nsc, FREE, KT, P).transpose(0, 3, 2, 1).astype(bf16))
    return {
        "xh": xh,
        "vch": np.ascontiguousarray(vc.astype(bf16)),
        "vdh": np.ascontiguousarray(vd.astype(bf16)),
    }


def make_in_maps(inputs):
    B = np.asarray(inputs["h_final"]).shape[0]
    bc = B // NCORES
    shared = pack_weights(inputs)
    hf = np.asarray(inputs["h_final"], dtype=np.float32)
    vc = np.asarray(inputs["v_claim"], dtype=np.float32)
    vd = np.asarray(inputs["v_doc"], dtype=np.float32)
    in_maps = []
    for c in range(NCORES):
        sl = slice(c * bc, (c + 1) * bc)
        m = dict(shared)
        m.update(pack_core(hf[sl], vc[sl], vd[sl]))
        in_maps.append(m)
    return in_maps


def kernel(**inputs):
    _shim_axon_hooks()
    from concourse.bass_utils import run_bass_kernel_spmd

    bc = batch_per_core(inputs)
    nc = get_nc(bc)
    in_maps = make_in_maps(inputs)
    res = run_bass_kernel_spmd(nc, in_maps, core_ids=list(range(NCORES)))
    return np.concatenate([r["out"] for r in res.results], axis=0)
